# revision 4
# baseline (speedup 1.0000x reference)
"""Trainium2 Bass kernel v5: single-head causal attention (B=8, T=2048, D=1024, HS=64).

Sharding: data-parallel over batch B -- one batch element per NeuronCore (8 cores).

v5 design (vs the 83us v3 baseline):
  * x arrives t-block-major; block 0 lands in 4 pair-sized sub-DMAs so the
    first projection matmul starts ~2us after the DMAs begin instead of
    waiting for the full 4MB.  Projections chase the DMA stream.
  * All matmuls bf16 (fp8 was measured at 5.9e-2 rel err -- softmax weight
    errors pass through multiplicatively, so the ~2^-8 of bf16 is required).
  * S matmuls contract over only h=64, so two chunks run CONCURRENTLY in the
    PE array via row-band packing: Q.T/K.T are stored twice (partitions 0-63
    and 64-127); consecutive chunk matmuls land in different row groups and
    overlap, halving S time.
  * S/exp work in chunk pairs sharing a 2-bank PSUM tile: one ACTIVATE covers
    1024 columns (ScalarE exp is the system floor; fewer instructions).
  * Causal masking on GpSimd (SBUF-only engine, otherwise idle).
  * O.T accumulates [h(64) | denom] x t in PSUM via a 32.0-column in Vn (the
    weight pre-scale x32 keeps everything consistent); each block's [65, 512]
    PSUM tile DMAs straight to DRAM.  The host performs denom divide + final
    transpose while unsharding.  No finalize chain -> no cross-queue
    head-of-line stalls, PE never idles on a DVE reciprocal.
"""
import os
import sys

for _p in ("/opt/trn_rl_repo", "/root/.axon_site/_ro/trn_rl_repo"):
    if _p not in sys.path and os.path.isdir(_p):
        sys.path.append(_p)

import numpy as np
import jax

try:
    jax.config.update("jax_compilation_cache_dir", "/tmp/jax_neff_cache")
    jax.config.update("jax_persistent_cache_min_compile_time_secs", 1.0)
    jax.config.update("jax_persistent_cache_min_entry_size_bytes", -1)
except Exception:
    pass

import concourse.mybir as mybir
import concourse.tile as tile
from concourse import bacc
from concourse.bass_utils import run_bass_kernel_spmd
from concourse.masks import make_identity

B, T, D, HS = 8, 2048, 1024, 64
NCORES = 8
QB = 512            # query block (free dim of S.T tiles)
NQB = T // QB       # 4
NKC = T // 128      # 16 key chunks
ND = D // 128       # 8 contraction chunks
WS = 32.0           # weight pre-scale (power of 2; exact, folded into exp scale)

MM_MODE = os.environ.get("BASS_MM_MODE", "bf16")
FALLBACK_MODE = "bf16"

F32 = mybir.dt.float32
BF16 = mybir.dt.bfloat16


def build(mode=None):
    nc = bacc.Bacc(None)
    # x.T packed t-block-major: xTp[p, j*(ND*QB) + dc*QB + q] = x.T[dc*128+p, j*QB+q]
    xTp = nc.declare_dram_parameter("xTp", [128, NQB * ND * QB], BF16, isOutput=False)
    # weights: wqk.T image [ND,128] then wv.T image [ND,64], pre-scaled x32
    wpk = nc.declare_dram_parameter("wpk", [128, ND * 192], BF16, isOutput=False)
    # col 0: 32*qkb stacked [q;k]; col 1 rows 0:64: 32*vb
    bias32 = nc.declare_dram_parameter("bias32", [128, 2], F32, isOutput=False)
    # unnormalized O.T (rows 0:64) + softmax denominator (row 64)
    outT = nc.declare_dram_parameter("outT", [HS + 1, T], F32, isOutput=True)

    scale = float(1.0 / (np.sqrt(HS) * WS * WS))

    with tile.TileContext(nc) as tc:
        with tc.tile_pool(name="const", bufs=1) as cpool, \
             tc.tile_pool(name="big", bufs=1) as bpool, \
             tc.tile_pool(name="pex", bufs=14) as ppool:

            wqk_t = cpool.tile([128, ND, 128], BF16, tag="wqk")
            wv_t = cpool.tile([128, ND, 64], BF16, tag="wv")
            bias_t = cpool.tile([128, 2], F32, tag="bias")
            xTs = bpool.tile([128, NQB, ND, QB], BF16, tag="xTs")

            # ---- DMAs: the QK weight image and x block 0 gate the first
            # matmuls -- they go first on the two HW rings.  The V image and
            # bias are needed only mid-block.  Blocks 1-3 are issued lazily
            # from inside stage_gen so early HBM bandwidth goes to block 0.
            nc.scalar.dma_start(wqk_t[:], wpk[:, 0:ND * 128])
            for i in range(4):
                q = nc.sync if i % 2 == 0 else nc.scalar
                q.dma_start(xTs[:, 0, 2 * i:2 * i + 2, :],
                            xTp[:, i * 1024:(i + 1) * 1024])
            nc.sync.dma_start(wv_t[:], wpk[:, ND * 128:ND * 192])
            nc.scalar.dma_start(bias_t[:], bias32[:])

            def load_x_block(j):
                q = (nc.scalar, nc.sync, nc.scalar)[j - 1]
                q.dma_start(xTs[:, j, :, :],
                            xTp[:, j * ND * QB:(j + 1) * ND * QB])

            # ---- constants (identity first: it gates the PE warmups) ----
            id_32 = cpool.tile([128, 128], F32, tag="id_32")
            make_identity(nc, id_32[:])
            idb = cpool.tile([128, 128], BF16, tag="idb")
            nc.vector.tensor_copy(idb[:], id_32[:])
            # 0/1 lower-causal mask (keep iff col >= partition)
            trim = cpool.tile([128, 128], F32, tag="trim")
            nc.gpsimd.memset(trim[:], 1.0)
            nc.gpsimd.affine_select(
                out=trim[:], in_=trim[:],
                compare_op=mybir.AluOpType.is_ge,
                fill=0.0, base=0,
                pattern=[[1, 128]], channel_multiplier=-1)
            trimb = cpool.tile([128, 128], BF16, tag="trimb")
            nc.vector.tensor_copy(trimb[:], trim[:])

            # Q.T/K.T duplicated on both partition halves for row-band packing
            QTd = bpool.tile([128, T], BF16, tag="QTd")
            KTd = bpool.tile([128, T], BF16, tag="KTd")
            VTr = bpool.tile([64, T], BF16, tag="VTr")
            Vn = bpool.tile([128, NKC, HS + 1], BF16, tag="Vn")
            OT_sb = bpool.tile([HS + 1, T], F32, tag="OT")
            nc.gpsimd.memset(Vn[:, :, HS:HS + 1], WS)  # denom col = 32.0

            with tc.tile_pool(name="psS", bufs=2, space="PSUM") as psS, \
                 tc.tile_pool(name="psO", bufs=2, space="PSUM") as psO:

                # warm the PE while x block 0 lands: raises the HAM duty
                # cycle so the projections start at full clock
                wu = psO.tile([128, 128], F32, tag="aux", bufs=1)
                for _ in range(12):
                    nc.tensor.transpose(wu[:], id_32[:], id_32[:])

                pexp_by = {j: {} for j in range(NQB)}

                def s_pair(j, q):
                    """One S chunk-pair: two row-band-packed matmuls into a
                    2-bank PSUM tile, exp to bf16, causal masks on Pool."""
                    c0, c1 = 2 * q, 2 * q + 1
                    f0a = max(0, 128 * (c0 - 4 * j))
                    f0b = max(0, 128 * (c1 - 4 * j))
                    sp = psS.tile([128, 1024], F32, tag="sp")
                    nc.tensor.matmul(sp[:, f0a:512],
                                     KTd[0:64, c0 * 128:(c0 + 1) * 128],
                                     QTd[0:64, j * QB + f0a:(j + 1) * QB],
                                     start=True, stop=True)
                    nc.tensor.matmul(sp[:, 512 + f0b:1024],
                                     KTd[64:128, c1 * 128:(c1 + 1) * 128],
                                     QTd[64:128, j * QB + f0b:(j + 1) * QB],
                                     start=True, stop=True)
                    pe_t = ppool.tile([128, 1024], BF16, tag="pe")
                    if f0a == f0b:
                        nc.scalar.activation(pe_t[:, 0:1024], sp[:, 0:1024],
                                             mybir.ActivationFunctionType.Exp,
                                             scale=scale)
                    else:
                        nc.scalar.activation(pe_t[:, f0a:512], sp[:, f0a:512],
                                             mybir.ActivationFunctionType.Exp,
                                             scale=scale)
                        nc.scalar.activation(pe_t[:, 512 + f0b:1024],
                                             sp[:, 512 + f0b:1024],
                                             mybir.ActivationFunctionType.Exp,
                                             scale=scale)
                    if c0 >= 4 * j:
                        nc.gpsimd.tensor_mul(pe_t[:, f0a:f0a + 128],
                                             pe_t[:, f0a:f0a + 128], trimb[:])
                    if c1 >= 4 * j:
                        nc.gpsimd.tensor_mul(
                            pe_t[:, 512 + f0b:512 + f0b + 128],
                            pe_t[:, 512 + f0b:512 + f0b + 128], trimb[:])
                    pexp_by[j][q] = (pe_t, f0a, f0b)

                def stage_gen(j):
                    """Projections + V naturalization + S/exp for query block
                    j; yields between instruction groups so the driver weaves
                    this between block j-1's O matmuls.  Vn copies are
                    emitted late so the DVE queue head stays available for
                    ops that feed the PE."""
                    jsl = slice(j * QB, (j + 1) * QB)
                    if j + 1 < NQB:
                        load_x_block(j + 1)
                    ps = psS.tile([128, 512], F32, tag="pj", bufs=1)
                    for dc in range(ND):
                        nc.tensor.matmul(ps[:, 0:512], wqk_t[:, dc, :],
                                         xTs[:, j, dc, :],
                                         start=(dc == 0), stop=(dc == ND - 1))
                        if dc % 2 == 1:
                            if j == 0 and dc < 7:
                                # block 0 chases the x DMA; these fills keep
                                # the PE (and HAM's activity window) busy
                                # during the transfer stalls
                                for _ in range(6):
                                    nc.tensor.transpose(wu[:], id_32[:], id_32[:])
                            yield
                    # bias-add copies, both partition halves (row-band packing)
                    nc.vector.tensor_scalar_add(QTd[0:64, jsl], ps[0:64, 0:512],
                                                bias_t[0:64, 0:1])
                    nc.vector.tensor_scalar_add(KTd[0:64, jsl], ps[64:128, 0:512],
                                                bias_t[64:128, 0:1])
                    yield
                    nc.vector.tensor_scalar_add(QTd[64:128, jsl], ps[0:64, 0:512],
                                                bias_t[0:64, 0:1])
                    nc.vector.tensor_scalar_add(KTd[64:128, jsl], ps[64:128, 0:512],
                                                bias_t[64:128, 0:1])
                    yield
                    # all S pairs for this block (the exp stream is the
                    # critical path of the final block -- start it early)
                    for q in range(2 * j + 2):
                        s_pair(j, q)
                        yield
                    # V projection into its own PSUM tile
                    pV = psO.tile([64, 512], F32, tag="aux", bufs=1)
                    for dc in range(ND):
                        nc.tensor.matmul(pV[:], wv_t[:, dc, :],
                                         xTs[:, j, dc, :],
                                         start=(dc == 0), stop=(dc == ND - 1))
                        if dc % 2 == 1:
                            yield
                    # V bias rides the PSUM->SBUF copy (per-partition in V.T)
                    nc.vector.tensor_scalar_add(VTr[:, jsl], pV[:],
                                                bias_t[0:64, 1:2])
                    yield
                    pt = psO.tile([128, 4, HS], BF16, tag="aux", bufs=1)
                    for r in range(4):
                        c = 4 * j + r
                        nc.tensor.transpose(pt[:, r, :],
                                            VTr[:, c * 128:(c + 1) * 128],
                                            idb[0:64, 0:64])
                        yield
                    nc.vector.tensor_copy(Vn[:, 4 * j:4 * j + 4, 0:HS], pt[:])
                    yield

                # block 0 runs eagerly (nothing to weave it into)
                for _ in stage_gen(0):
                    pass

                for j in range(NQB):
                    gen = stage_gen(j + 1) if j + 1 < NQB else None

                    def adv(n=1):
                        nonlocal gen
                        if gen is None:
                            return
                        for _ in range(n):
                            try:
                                next(gen)
                            except StopIteration:
                                gen = None
                                return

                    jsl = slice(j * QB, (j + 1) * QB)
                    ncl = 4 * j + 4
                    po = psO.tile([HS + 1, 512], F32, tag="po")
                    for c in range(ncl):
                        pe_t, f0a, f0b = pexp_by[j][c // 2]
                        f0 = f0a if c % 2 == 0 else f0b
                        off = 0 if c % 2 == 0 else 512
                        nc.tensor.matmul(po[:, f0:512], Vn[:, c, :],
                                         pe_t[:, off + f0:off + 512],
                                         start=(c == 0), stop=(c == ncl - 1))
                        adv(2)
                    # O.T + denom row to DRAM via SBUF bounce; host normalizes
                    h0, h1 = j * QB, j * QB + 256
                    nc.vector.tensor_copy(OT_sb[:, h0:h0 + 256], po[:, 0:256])
                    nc.sync.dma_start(outT[:, h0:h0 + 256], OT_sb[:, h0:h0 + 256])
                    nc.vector.tensor_copy(OT_sb[:, h1:h1 + 256], po[:, 256:512])
                    nc.sync.dma_start(outT[:, h1:h1 + 256], OT_sb[:, h1:h1 + 256])
                    while gen is not None:
                        adv(1)

    nc.compile()
    return nc


_RUNNERS = {}


def _get_runner(mode=None):
    mode = mode or MM_MODE
    if mode not in _RUNNERS:
        _RUNNERS[mode] = build(mode)
    return _RUNNERS[mode]


def make_in_maps(x, wq_w, wq_b, wk_w, wk_b, wv_w, wv_b, mode=None):
    import ml_dtypes
    hd = ml_dtypes.bfloat16
    x = np.asarray(x, np.float32)
    wqk = np.concatenate([np.asarray(wq_w, np.float32),
                          np.asarray(wk_w, np.float32)], axis=0) * WS  # [128, D]
    wv = np.asarray(wv_w, np.float32) * WS                             # [64, D]
    wpk = np.zeros((128, ND * 192), np.float32)
    for dc in range(ND):
        wpk[:, dc * 128:(dc + 1) * 128] = wqk[:, dc * 128:(dc + 1) * 128].T
        wpk[:, ND * 128 + dc * 64:ND * 128 + (dc + 1) * 64] = \
            wv[:, dc * 128:(dc + 1) * 128].T
    wpk = np.ascontiguousarray(wpk).astype(hd)
    bias32 = np.zeros((128, 2), np.float32)
    bias32[:, 0] = np.concatenate([np.asarray(wq_b, np.float32),
                                   np.asarray(wk_b, np.float32)]) * WS
    bias32[0:64, 1] = np.asarray(wv_b, np.float32) * WS
    in_maps = []
    for b in range(B):
        xT = x[b].T  # [D, T]
        xTp = np.ascontiguousarray(
            xT.reshape(ND, 128, NQB, QB).transpose(1, 2, 0, 3).reshape(128, -1)
        ).astype(hd)
        in_maps.append({"xTp": xTp, "wpk": wpk, "bias32": bias32})
    return in_maps


def run(in_maps, trace=False, tmpdir=None, mode=None):
    nc = _get_runner(mode)
    return run_bass_kernel_spmd(nc, in_maps, core_ids=list(range(NCORES)),
                                trace=trace, tmpdir=tmpdir)


def _assemble(res):
    outs = []
    for b in range(B):
        o = np.asarray(res.results[b]["outT"], np.float32)   # [65, T]
        outs.append((o[0:HS, :] / o[HS:HS + 1, :]).T)        # normalize + transpose
    return np.stack(outs, axis=0)


def _canary_ok(out, x, wq_w, wq_b, wk_w, wk_b, wv_w, wv_b):
    """Cheap exact check of causal rows t=0,1 (closed-form, tiny host cost)."""
    x2 = np.asarray(x, np.float32)[:, 0:2, :].astype(np.float64)      # [B,2,D]
    q = x2 @ np.asarray(wq_w, np.float64).T + np.asarray(wq_b, np.float64)
    k = x2 @ np.asarray(wk_w, np.float64).T + np.asarray(wk_b, np.float64)
    v = x2 @ np.asarray(wv_w, np.float64).T + np.asarray(wv_b, np.float64)
    exp0 = v[:, 0, :]                                                 # [B,HS]
    s = np.einsum("bh,bsh->bs", q[:, 1, :], k) / np.sqrt(HS)          # [B,2]
    w = np.exp(s - s.max(-1, keepdims=True))
    w = w / w.sum(-1, keepdims=True)
    exp1 = np.einsum("bs,bsh->bh", w, v)
    got = np.stack([out[:, 0, :], out[:, 1, :]], axis=1)
    want = np.stack([exp0, exp1], axis=1)
    rel = np.abs(got - want) / max(np.abs(want).max(), 1e-6)
    return np.isfinite(got).all() and rel.max() < 3e-2


def kernel(x, wq_w, wq_b, wk_w, wk_b, wv_w, wv_b):
    args = (x, wq_w, wq_b, wk_w, wk_b, wv_w, wv_b)
    res = run(make_in_maps(*args, mode=MM_MODE), mode=MM_MODE)
    return _assemble(res)
